# revision 30
# baseline (speedup 1.0000x reference)
# NonLocalBlock Trainium2 Bass kernel (v6).
#
# Reference computation (per batch b):
#   theta = theta_w @ X + theta_b          [IC, N]   (X = x[b] as [C, N])
#   phi   = phi_w   @ X + phi_b            [IC, N]
#   g     = g_w     @ X + g_b              [IC, N]
#   attn  = softmax_j(theta^T phi)         [N, N]
#   att   = g @ attn^T                     [IC, N]
#   y     = BN(w_w @ att + w_b) + x
#
# Math folds used on device (validated vs reference):
#   - phi bias drops out of softmax entirely (adds an i-only constant).
#   - g bias folds into the final bias because attn rows sum to 1.
#   - BN is affine: fold into w_eff = inv*w_w and b_final.
#   - scores bounded (|s| < 50) so exp() needs no max-subtraction.
#
# Sharding: 8 cores = 4 batches x 2 row-halves (pure SPMD).
#
# Pipeline structure (v6):
#   - x and all projection weights ship as fp16: half the HBM traffic
#     (x = 2MB in 8 big DMAs), FWL on every weight load, and |x| < 6 so
#     fp16 rounding (~5e-4) is negligible against the 2e-2 gate.
#   - scores are computed transposed ([j,i]) in fp16 (phi stationary,
#     theta moving) so exp feeds att = g @ attn^T directly.
#   - exp is written as bf16; the softmax denominator is accumulated by
#     the DVE (one [128,2048] bf16 add per exp tile, 2 elem/cyc/lane),
#     folded to one chunk-slot, then a single ones-matmul per block does
#     the cross-partition reduction. The last block instead feeds its
#     final exp tile straight into ones-matmuls and runs a half-width
#     normalize/W/store pipeline to shorten the drain.
#   - AV consumption runs through a global 4-group-deferred queue that
#     crosses block boundaries, so block-end leftovers interleave with
#     the next block's scores instead of starving the ACT engine.
#   - gT is produced directly as x_chunk^T @ g_w^T, emitted in the same
#     slice as its projection (same-slice PSUM rotation, no convoy).
#   - The exp-table preload is emitted before the late x DMA triggers on
#     the ACT ring so the table is resident before the first real exp.
#   - PSUM: sc 2x[128,1024]=4 banks, att 2x[128,512], pp 2x[128,512].

from contextlib import ExitStack

import numpy as np

import concourse.bass as bass
import concourse.tile as tile
from concourse import bacc, mybir
from concourse.bass_utils import run_bass_kernel_spmd

F32 = mybir.dt.float32
F32R = mybir.dt.float32r
BF16 = mybir.dt.bfloat16
F16 = mybir.dt.float16
AF = mybir.ActivationFunctionType

B, C, IC = 4, 256, 128
H = W = 64
N = H * W            # 4096
HALF = N // 2        # 2048 rows of attention per core
P = 128
NCORES = 8
NBLK = HALF // 512   # 4 i-blocks of 512
NCH = N // P         # 32 j-chunks of 128
NGRP = NCH // 2      # 16 groups of 2 chunks per i-block
NPAIR = NGRP // 2    # 8 exp-tile pairs per i-block
DEFER = 4            # consume exp output this many groups late (global)
BN_EPS = 1e-5


def _emit_consume(nc, pools, blk, grp):
    """AV matmuls for group `grp` of block `blk` (bf16)."""
    att_ps = pools["att_ps"][blk]
    gT_bf = pools["gT_bf"]
    ex2 = pools["ex_sbs"][(blk, grp // 2)]
    off = (grp % 2) * 1024
    for c in range(2):
        jc = grp * 2 + c
        nc.tensor.matmul(
            att_ps[:], gT_bf[:, jc * P:(jc + 1) * P],
            ex2[:, off + c * 512:off + (c + 1) * 512],
            start=jc == 0, stop=jc == NCH - 1)


def _emit_den_fold(nc, pools, blk):
    """Fold acc's 4 chunk-slots down to one (slot 0) on the DVE."""
    acc = pools["acc"][blk]
    nc.vector.tensor_add(acc[:, 0:1024], acc[:, 0:1024], acc[:, 1024:2048])
    nc.vector.tensor_add(acc[:, 0:512], acc[:, 0:512], acc[:, 512:1024])


def _emit_den_tt(nc, pools, blk, pair):
    """Accumulate one exp tile (2 groups) into the block's bf16 acc."""
    acc = pools["acc"][blk]
    ex2 = pools["ex_sbs"][(blk, pair)]
    if pair == 0:
        nc.vector.tensor_copy(acc[:], ex2[:])
    else:
        nc.vector.tensor_add(acc[:], acc[:], ex2[:])
    last = NBLK - 1
    if (blk < last and pair == NPAIR - 1) or (blk == last and
                                              pair == NPAIR - 3):
        _emit_den_fold(nc, pools, blk)


def _flush_tt(nc, pools):
    for blk, pair in pools["pend_tt"]:
        _emit_den_tt(nc, pools, blk, pair)
    pools["pend_tt"] = []


def _pop_av(nc, pools, yout):
    blk, grp = pools["pend_av"].pop(0)
    _emit_consume(nc, pools, blk, grp)
    if grp == NGRP - 1:
        pools["tail_due"].append(blk)


def _emit_due_tails(nc, pools, yout):
    for blk in pools["tail_due"]:
        _emit_block_tail(nc, pools, blk, yout)
    pools["tail_due"] = []


def _emit_group(nc, pools, blk, grp, yout, defer_tt=False):
    """AV (deferred), then scores + exp for one [128,1024] group."""
    ps_pool, ex_pool, acc_pool = pools["ps"], pools["ex"], pools["accp"]
    theta_sb, phi_sb = pools["theta_sb"], pools["phi_sb"]
    isl = slice(blk * 512, (blk + 1) * 512)
    if grp == 0:
        pools["att_ps"][blk] = ps_pool.tile(
            [P, 512], F32, name=f"att_ps{blk}", tag="att", bufs=2)
        pools["acc"][blk] = acc_pool.tile(
            [P, 2048], BF16, name=f"acc{blk}", tag="acc", bufs=2)
    if grp % 2 == 0:
        pools["ex_sbs"][(blk, grp // 2)] = ex_pool.tile(
            [P, 2048], BF16, name=f"ex{blk}_{grp // 2}", tag="ex", bufs=8)
    if len(pools["pend_av"]) >= DEFER:
        _pop_av(nc, pools, yout)
    _emit_due_tails(nc, pools, yout)
    sc_ps = ps_pool.tile([P, 1024], F32, name=f"sc{blk}_{grp}", tag="sc",
                         bufs=2)
    for c in range(2):
        jc = grp * 2 + c
        nc.tensor.matmul(
            sc_ps[:, c * 512:(c + 1) * 512],
            phi_sb[:, jc * P:(jc + 1) * P],
            theta_sb[:, isl],
            start=True, stop=True)
    ex2 = pools["ex_sbs"][(blk, grp // 2)]
    off = (grp % 2) * 1024
    nc.scalar.activation(ex2[:, off:off + 1024], sc_ps[:], AF.Exp)
    pools["pend_av"].append((blk, grp))
    # last block: pairs 6 and 7 skip the DVE accumulate; the drain feeds
    # them straight into the denominator ones-matmuls
    if grp % 2 == 1 and not (blk == NBLK - 1 and grp >= NGRP - 4):
        if defer_tt:
            pools["pend_tt"].append((blk, grp // 2))
        else:
            _emit_den_tt(nc, pools, blk, grp // 2)


def _emit_tail_finish(nc, pools, blk, yout, den_ps):
    """Reciprocal, normalize, W projection, bias+residual, store."""
    ps_pool, rec_pool = pools["ps"], pools["rec"]
    wef_hf, xb_sb = pools["wef_hf"], pools["xb_sb"]
    att_ps = pools["att_ps"][blk]
    last = blk == NBLK - 1

    recb = rec_pool.tile([P, 512], F32, name=f"recb{blk}", tag="recb")
    nc.vector.reciprocal_approx_fast(out=recb[:], in_=den_ps[:])
    halves = (slice(0, 256), slice(256, 512)) if last else (slice(0, 512),)
    attn_sb = rec_pool.tile([P, 512], F16, name=f"attn{blk}", tag="attn")
    for h, hsl in enumerate(halves):
        isl = slice(blk * 512 + hsl.start, blk * 512 + hsl.stop)
        nc.vector.tensor_mul(attn_sb[:, hsl], att_ps[:, hsl], recb[:, hsl])
        for k in range(2):
            y_ps = ps_pool.tile([P, 512], F32, name=f"y{blk}_{h}_{k}",
                                tag="pp", bufs=2)
            nc.tensor.matmul(
                y_ps[:, hsl], wef_hf[:, k * P:(k + 1) * P], attn_sb[:, hsl],
                start=True, stop=True)
            yo = rec_pool.tile([P, 512], F32, name=f"yo{blk}_{h}_{k}",
                               tag="yo", bufs=4)
            nc.vector.tensor_add(yo[:, hsl], y_ps[:, hsl], xb_sb[k][:, isl])
            # ACT-ring triggers are only free once the exp stream is done
            eng = nc.scalar if (last and k == 1) else nc.sync
            eng.dma_start(out=yout[k * P:(k + 1) * P, isl], in_=yo[:, hsl])


def _emit_block_tail(nc, pools, blk, yout):
    ps_pool = pools["ps"]
    onesP_bf = pools["onesP_bf"]
    acc = pools["acc"][blk]
    den_ps = ps_pool.tile([P, 512], F32, name=f"den_ps{blk}", tag="pp",
                          bufs=2)
    nc.tensor.matmul(den_ps[:], onesP_bf[:], acc[:, 0:512],
                     start=True, stop=True)
    _emit_tail_finish(nc, pools, blk, yout, den_ps)


def _kernel_body(ctx, tc, ins, yout):
    nc = tc.nc
    xin, thw, phw, gw, wef, tb, bfin = (
        ins["xin"], ins["thw"], ins["phw"], ins["gw"], ins["wef"],
        ins["tb"], ins["bfin"])

    consts = ctx.enter_context(tc.tile_pool(name="consts", bufs=1))
    big = ctx.enter_context(tc.tile_pool(name="big", bufs=1))

    # ---- x load: fp16, 1024-col double-slices, both HWDGE rings ----
    x_sb = [big.tile([P, N], F16, name=f"x_sb{k}") for k in range(2)]

    def xdma(T, k):
        tsl = slice(T * 1024, (T + 1) * 1024)
        eng = nc.sync if k == 0 else nc.scalar
        eng.dma_start(out=x_sb[k][:, tsl], in_=xin[k * P:(k + 1) * P, tsl])

    # projection weights go first as small pieces on the ACT ring (they
    # gate the first matmul); x cols 0-511 go as 256-col pieces across
    # queues (per-queue BW ~22GB/s makes piece size the landing time)
    thw_sb = consts.tile([P, C], F16, name="thw_sb")
    phw_sb = consts.tile([P, C], F16, name="phw_sb")
    gw_sb = [consts.tile([P, IC], F16, name=f"gw_sb{k}") for k in range(2)]
    tb_sb = consts.tile([P, 1], F32, name="tb_sb")
    for k in range(2):
        nc.scalar.dma_start(out=thw_sb[:, k * P:(k + 1) * P],
                            in_=thw[k * P:(k + 1) * P, :])
    for q in range(2):
        for k in range(2):
            tsl = slice(q * 256, (q + 1) * 256)
            eng = nc.sync if k == 0 else nc.scalar
            eng.dma_start(out=x_sb[k][:, tsl],
                          in_=xin[k * P:(k + 1) * P, tsl])
    for k in range(2):
        nc.sync.dma_start(out=gw_sb[k][:], in_=gw[k * P:(k + 1) * P, :])
        nc.scalar.dma_start(out=phw_sb[:, k * P:(k + 1) * P],
                            in_=phw[k * P:(k + 1) * P, :])
    nc.sync.dma_start(out=tb_sb[:], in_=tb[:, None])
    for k in range(2):
        tsl = slice(512, 1024)
        eng = nc.sync if k == 0 else nc.scalar
        eng.dma_start(out=x_sb[k][:, tsl], in_=xin[k * P:(k + 1) * P, tsl])
    # preload the exp table set before the late x triggers queue up on
    # the ACT ring; tb_sb is already in flight on this ring
    exdum = consts.tile([P, 1], F32, name="exdum")
    nc.scalar.activation(exdum[:], tb_sb[:], AF.Exp)
    wef_hf = consts.tile([P, C], F16, name="wef_hf")
    nc.sync.dma_start(out=wef_hf[:], in_=wef[:, :])
    bfin_sb = consts.tile([P, 2], F32, name="bfin_sb")
    nc.sync.dma_start(out=bfin_sb[:], in_=bfin.rearrange("(k p) -> p k", p=P))
    for T in range(1, 4):
        for k in range(2):
            xdma(T, k)
    onesP_bf = consts.tile([P, P], BF16, name="onesP_bf")
    nc.vector.memset(onesP_bf[:], 1.0)

    theta_sb = big.tile([P, HALF], F16, name="theta_sb")
    phi_sb = big.tile([P, N], F16, name="phi_sb")
    gT_bf = big.tile([P, N], BF16, name="gT_bf")
    xb_sb = [big.tile([P, HALF], F16, name=f"xb_sb{k}") for k in range(2)]

    # ---- single PSUM pool, tagged slots (8 banks total):
    #   sc 2x[128,1024]=4, att 2x[128,512]=2, pp 2x[128,512]=2
    ps_pool = ctx.enter_context(tc.tile_pool(name="ps", bufs=1, space="PSUM"))
    pools = {
        "ps": ps_pool,
        "ex": ctx.enter_context(tc.tile_pool(name="ex", bufs=8)),
        "accp": ctx.enter_context(tc.tile_pool(name="accp", bufs=2)),
        "rec": ctx.enter_context(tc.tile_pool(name="rec", bufs=2)),
        "theta_sb": theta_sb, "phi_sb": phi_sb, "gT_bf": gT_bf,
        "onesP_bf": onesP_bf, "wef_hf": wef_hf, "xb_sb": xb_sb,
        "att_ps": {}, "acc": {}, "ex_sbs": {},
        "pend_tt": [], "pend_av": [], "tail_due": [],
    }

    def gt_chunks(t):
        # 4 chunks land side-by-side in one PSUM tile -> one cast per slice
        pst = ps_pool.tile([P, 512], F32, name=f"gt_ps{t}", tag="pp",
                           bufs=2)
        for i in range(4):
            jc = 4 * t + i
            jsl = slice(jc * P, (jc + 1) * P)
            for k in range(2):
                nc.tensor.matmul(pst[:, i * P:(i + 1) * P],
                                 x_sb[k][:, jsl], gw_sb[k][:],
                                 start=k == 0, stop=k == 1)
        nc.vector.tensor_copy(gT_bf[:, t * 512:(t + 1) * 512], pst[:])

    def proj(t):
        # interleave the k-halves of theta/phi (separate PSUM banks
        # accumulate concurrently) so each x half is consumed as it lands
        tsl = slice(t * 512, (t + 1) * 512)
        th_ps = None
        if t < NBLK:
            th_ps = ps_pool.tile([P, 512], F32, name=f"th_ps{t}", tag="pp",
                                 bufs=2)
        ph_ps = ps_pool.tile([P, 512], F32, name=f"ph_ps{t}", tag="pp",
                             bufs=2)
        for k in range(2):
            if th_ps is not None:
                nc.tensor.matmul(th_ps[:], thw_sb[:, k * P:(k + 1) * P],
                                 x_sb[k][:, tsl],
                                 start=(k == 0), stop=(k == 1))
            nc.tensor.matmul(ph_ps[:], phw_sb[:, k * P:(k + 1) * P],
                             x_sb[k][:, tsl],
                             start=(k == 0), stop=(k == 1))
        if th_ps is not None:
            nc.vector.tensor_scalar_add(theta_sb[:, tsl], th_ps[:], tb_sb[:])
        nc.vector.tensor_copy(phi_sb[:, tsl], ph_ps[:])

    # ---- phase 1: slice-pipelined projections + gT (same-slice PSUM
    # rotation), interleaved with blocks 0 AND 1 of the attention (4
    # groups per x slice) so the PE is fed as soon as each slice lands.
    # block 0's groups for slice t are emitted right after slice t's
    # proj/gt and BEFORE slice t+1's, so the first scores only wait on x
    # cols 0-511; block 1 lags one slice (its theta needs proj(1)).
    for t in range(8):
        proj(t)
        for blkg, tg in ((0, t), (1, t - 1)):
            for gg in (2 * tg, 2 * tg + 1):
                if 0 <= gg < 14:
                    _emit_group(nc, pools, blkg, gg, yout, defer_tt=True)
        gt_chunks(t)
        _flush_tt(nc, pools)
    _flush_tt(nc, pools)
    for grp in range(14, NGRP):
        _emit_group(nc, pools, 0, grp, yout)
    for grp in range(14, NGRP):
        _emit_group(nc, pools, 1, grp, yout)
    for k in range(2):
        nc.vector.tensor_scalar_add(xb_sb[k][:], x_sb[k][:, 0:HALF],
                                    bfin_sb[:, k:k + 1])

    # ---- remaining i-blocks; AV consumption and tails flow through the
    # global deferred queue so block boundaries stay busy on both engines.
    for blk in range(2, NBLK):
        for grp in range(NGRP):
            _emit_group(nc, pools, blk, grp, yout)

    # drain: the last block's pairs 6/7 feed the denominator ones-matmuls
    # directly, interleaved with the leftover AVs in data-arrival order so
    # the reciprocal can start right after the final exp
    last = NBLK - 1
    ones = pools["onesP_bf"]
    ex6 = pools["ex_sbs"][(last, NPAIR - 2)]
    ex7 = pools["ex_sbs"][(last, NPAIR - 1)]
    den_ps = pools["ps"].tile([P, 512], F32, name=f"den_ps{last}",
                              tag="pp", bufs=2)
    _pop_av(nc, pools, yout)                       # AV(last, 12)
    nc.tensor.matmul(den_ps[:], ones[:], pools["acc"][last][:, 0:512],
                     start=True, stop=False)
    for c in range(4):                             # chunks of groups 12,13
        nc.tensor.matmul(den_ps[:], ones[:], ex6[:, c * 512:(c + 1) * 512],
                         start=False, stop=False)
    _pop_av(nc, pools, yout)                       # AV(last, 13)
    _pop_av(nc, pools, yout)                       # AV(last, 14)
    for c in range(2):                             # chunks of group 14
        nc.tensor.matmul(den_ps[:], ones[:], ex7[:, c * 512:(c + 1) * 512],
                         start=False, stop=False)
    _pop_av(nc, pools, yout)                       # AV(last, 15)
    for c in range(2, 4):                          # chunks of group 15
        nc.tensor.matmul(den_ps[:], ones[:], ex7[:, c * 512:(c + 1) * 512],
                         start=False, stop=c == 3)
    pools["tail_due"] = []
    _emit_tail_finish(nc, pools, last, yout, den_ps)


_CACHE = {}


def _build():
    if "nc" in _CACHE:
        return _CACHE["nc"]
    nc = bacc.Bacc("TRN2", target_bir_lowering=False, debug=False,
                   enable_asserts=False, num_devices=1)
    ins = {
        "xin": nc.dram_tensor("xin", [C, N], F16, kind="ExternalInput").ap(),
        "thw": nc.dram_tensor("thw", [C, IC], F16, kind="ExternalInput").ap(),
        "phw": nc.dram_tensor("phw", [C, IC], F16, kind="ExternalInput").ap(),
        "gw": nc.dram_tensor("gw", [C, IC], F16, kind="ExternalInput").ap(),
        "wef": nc.dram_tensor("wef", [IC, C], F16, kind="ExternalInput").ap(),
        "tb": nc.dram_tensor("tb", [IC], F32, kind="ExternalInput").ap(),
        "bfin": nc.dram_tensor("bfin", [C], F32, kind="ExternalInput").ap(),
    }
    yout = nc.dram_tensor("yout", [C, HALF], F32, kind="ExternalOutput").ap()
    with tile.TileContext(nc) as tc:
        with ExitStack() as ctx:
            _kernel_body(ctx, tc, ins, yout)
    nc.compile()
    _CACHE["nc"] = nc
    return nc


def _host_prepare(inputs):
    """Host-side folds + per-core input maps."""
    ii = {k: np.ascontiguousarray(np.asarray(v, dtype=np.float32))
          for k, v in inputs.items()}
    inv = ii["bn_gamma"] / np.sqrt(ii["bn_var"] + BN_EPS)
    w_eff = ii["w_w"] * inv[:, None]                       # [C, IC]
    b_final = (w_eff @ ii["g_b"] + ii["w_b"] * inv
               + ii["bn_beta"] - ii["bn_mean"] * inv)      # [C]
    f16 = np.float16
    shared = {
        "thw": np.ascontiguousarray(ii["theta_w"].T.astype(f16)),
        "phw": np.ascontiguousarray(ii["phi_w"].T.astype(f16)),
        "gw": np.ascontiguousarray(ii["g_w"].T.astype(f16)),
        "wef": np.ascontiguousarray(w_eff.T.astype(f16)),  # [IC, C]
        "tb": ii["theta_b"],
        "bfin": np.ascontiguousarray(b_final),
    }
    x = ii["x"].reshape(B, C, N)
    in_maps = []
    for core in range(NCORES):
        b, h = divmod(core, 2)
        own = x[b][:, h * HALF:(h + 1) * HALF]
        oth = x[b][:, (1 - h) * HALF:(2 - h) * HALF]
        xin = np.ascontiguousarray(
            np.concatenate([own, oth], axis=1).astype(f16))
        in_maps.append({"xin": xin, **shared})
    return in_maps


def _gather(results, x_dtype):
    out = np.empty((B, C, N), dtype=np.float32)
    for core in range(NCORES):
        b, h = divmod(core, 2)
        out[b][:, h * HALF:(h + 1) * HALF] = results[core]["yout"]
    return out.reshape(B, C, H, W).astype(x_dtype, copy=False)


def kernel(**inputs):
    nc = _build()
    in_maps = _host_prepare(inputs)
    res = run_bass_kernel_spmd(nc, in_maps, core_ids=list(range(NCORES)))
    return _gather(res.results, np.asarray(inputs["x"]).dtype)


# revision 31
# speedup vs baseline: 1.1898x; 1.1898x over previous
# NonLocalBlock Trainium2 Bass kernel (v6).
#
# Reference computation (per batch b):
#   theta = theta_w @ X + theta_b          [IC, N]   (X = x[b] as [C, N])
#   phi   = phi_w   @ X + phi_b            [IC, N]
#   g     = g_w     @ X + g_b              [IC, N]
#   attn  = softmax_j(theta^T phi)         [N, N]
#   att   = g @ attn^T                     [IC, N]
#   y     = BN(w_w @ att + w_b) + x
#
# Math folds used on device (validated vs reference):
#   - phi bias drops out of softmax entirely (adds an i-only constant).
#   - g bias folds into the final bias because attn rows sum to 1.
#   - BN is affine: fold into w_eff = inv*w_w and b_final.
#   - scores bounded (|s| < 50) so exp() needs no max-subtraction.
#
# Sharding: 8 cores = 4 batches x 2 row-halves (pure SPMD).
#
# Pipeline structure (v6):
#   - x and all projection weights ship as fp16: half the HBM traffic
#     (x = 2MB in 8 big DMAs), FWL on every weight load, and |x| < 6 so
#     fp16 rounding (~5e-4) is negligible against the 2e-2 gate.
#   - scores are computed transposed ([j,i]) in fp16 (phi stationary,
#     theta moving) so exp feeds att = g @ attn^T directly.
#   - exp is written as bf16; the softmax denominator is accumulated by
#     the DVE (one [128,2048] bf16 add per exp tile, 2 elem/cyc/lane),
#     folded to one chunk-slot, then a single ones-matmul per block does
#     the cross-partition reduction. The last block instead feeds its
#     final exp tile straight into ones-matmuls and runs a half-width
#     normalize/W/store pipeline to shorten the drain.
#   - AV consumption runs through a global 4-group-deferred queue that
#     crosses block boundaries, so block-end leftovers interleave with
#     the next block's scores instead of starving the ACT engine.
#   - gT is produced directly as x_chunk^T @ g_w^T, emitted in the same
#     slice as its projection (same-slice PSUM rotation, no convoy).
#   - The exp-table preload is emitted before the late x DMA triggers on
#     the ACT ring so the table is resident before the first real exp.
#   - PSUM: sc 2x[128,1024]=4 banks, att 2x[128,512], pp 2x[128,512].

from contextlib import ExitStack

import numpy as np

import concourse.bass as bass
import concourse.tile as tile
from concourse import bacc, mybir
from concourse.bass_utils import run_bass_kernel_spmd

F32 = mybir.dt.float32
F32R = mybir.dt.float32r
BF16 = mybir.dt.bfloat16
F16 = mybir.dt.float16
AF = mybir.ActivationFunctionType

B, C, IC = 4, 256, 128
H = W = 64
N = H * W            # 4096
HALF = N // 2        # 2048 rows of attention per core
P = 128
NCORES = 8
NBLK = HALF // 512   # 4 i-blocks of 512
NCH = N // P         # 32 j-chunks of 128
NGRP = NCH // 2      # 16 groups of 2 chunks per i-block
NPAIR = NGRP // 2    # 8 exp-tile pairs per i-block
DEFER = 4            # consume exp output this many groups late (global)
BN_EPS = 1e-5


def _emit_consume(nc, pools, blk, grp):
    """AV matmuls for group `grp` of block `blk` (bf16)."""
    att_ps = pools["att_ps"][blk]
    gT_bf = pools["gT_bf"]
    ex2 = pools["ex_sbs"][(blk, grp // 2)]
    off = (grp % 2) * 1024
    for c in range(2):
        jc = grp * 2 + c
        nc.tensor.matmul(
            att_ps[:], gT_bf[:, jc * P:(jc + 1) * P],
            ex2[:, off + c * 512:off + (c + 1) * 512],
            start=jc == 0, stop=jc == NCH - 1)


def _emit_den_fold(nc, pools, blk):
    """Fold acc's 4 chunk-slots down to one (slot 0) on the DVE."""
    acc = pools["acc"][blk]
    nc.vector.tensor_add(acc[:, 0:1024], acc[:, 0:1024], acc[:, 1024:2048])
    nc.vector.tensor_add(acc[:, 0:512], acc[:, 0:512], acc[:, 512:1024])


def _emit_den_tt(nc, pools, blk, pair):
    """Accumulate one exp tile (2 groups) into the block's bf16 acc."""
    acc = pools["acc"][blk]
    ex2 = pools["ex_sbs"][(blk, pair)]
    if pair == 0:
        nc.vector.tensor_copy(acc[:], ex2[:])
    else:
        nc.vector.tensor_add(acc[:], acc[:], ex2[:])
    last = NBLK - 1
    if (blk < last and pair == NPAIR - 1) or (blk == last and
                                              pair == NPAIR - 3):
        _emit_den_fold(nc, pools, blk)


def _flush_tt(nc, pools):
    for blk, pair in pools["pend_tt"]:
        _emit_den_tt(nc, pools, blk, pair)
    pools["pend_tt"] = []


def _pop_av(nc, pools, yout):
    blk, grp = pools["pend_av"].pop(0)
    _emit_consume(nc, pools, blk, grp)
    if grp == NGRP - 1:
        pools["tail_due"].append(blk)


def _emit_due_tails(nc, pools, yout):
    for blk in pools["tail_due"]:
        _emit_block_tail(nc, pools, blk, yout)
    pools["tail_due"] = []


def _emit_group(nc, pools, blk, grp, yout, defer_tt=False):
    """AV (deferred), then scores + exp for one [128,1024] group."""
    ps_pool, ex_pool, acc_pool = pools["ps"], pools["ex"], pools["accp"]
    theta_sb, phi_sb = pools["theta_sb"], pools["phi_sb"]
    isl = slice(blk * 512, (blk + 1) * 512)
    if grp == 0:
        pools["att_ps"][blk] = ps_pool.tile(
            [P, 512], F32, name=f"att_ps{blk}", tag="att", bufs=2)
        pools["acc"][blk] = acc_pool.tile(
            [P, 2048], BF16, name=f"acc{blk}", tag="acc", bufs=2)
    if grp % 2 == 0:
        pools["ex_sbs"][(blk, grp // 2)] = ex_pool.tile(
            [P, 2048], BF16, name=f"ex{blk}_{grp // 2}", tag="ex", bufs=8)
    if len(pools["pend_av"]) >= DEFER:
        _pop_av(nc, pools, yout)
    _emit_due_tails(nc, pools, yout)
    sc_ps = ps_pool.tile([P, 1024], F32, name=f"sc{blk}_{grp}", tag="sc",
                         bufs=2)
    for c in range(2):
        jc = grp * 2 + c
        nc.tensor.matmul(
            sc_ps[:, c * 512:(c + 1) * 512],
            phi_sb[:, jc * P:(jc + 1) * P],
            theta_sb[:, isl],
            start=True, stop=True)
    ex2 = pools["ex_sbs"][(blk, grp // 2)]
    off = (grp % 2) * 1024
    nc.scalar.activation(ex2[:, off:off + 1024], sc_ps[:], AF.Exp)
    pools["pend_av"].append((blk, grp))
    # last block: pairs 6 and 7 skip the DVE accumulate; the drain feeds
    # them straight into the denominator ones-matmuls
    if grp % 2 == 1 and not (blk == NBLK - 1 and grp >= NGRP - 4):
        if defer_tt:
            pools["pend_tt"].append((blk, grp // 2))
        else:
            _emit_den_tt(nc, pools, blk, grp // 2)


def _emit_tail_finish(nc, pools, blk, yout, den_ps):
    """Reciprocal, normalize, W projection, bias+residual, store."""
    ps_pool, rec_pool = pools["ps"], pools["rec"]
    wef_hf, xb_sb = pools["wef_hf"], pools["xb_sb"]
    att_ps = pools["att_ps"][blk]
    last = blk == NBLK - 1

    recb = rec_pool.tile([P, 512], F32, name=f"recb{blk}", tag="recb")
    nc.vector.reciprocal_approx_fast(out=recb[:], in_=den_ps[:])
    halves = (slice(0, 256), slice(256, 512)) if last else (slice(0, 512),)
    attn_sb = rec_pool.tile([P, 512], F16, name=f"attn{blk}", tag="attn")
    for h, hsl in enumerate(halves):
        isl = slice(blk * 512 + hsl.start, blk * 512 + hsl.stop)
        nc.vector.tensor_mul(attn_sb[:, hsl], att_ps[:, hsl], recb[:, hsl])
        for k in range(2):
            y_ps = ps_pool.tile([P, 512], F32, name=f"y{blk}_{h}_{k}",
                                tag="pp", bufs=2)
            nc.tensor.matmul(
                y_ps[:, hsl], wef_hf[:, k * P:(k + 1) * P], attn_sb[:, hsl],
                start=True, stop=True)
            yo = rec_pool.tile([P, 512], F32, name=f"yo{blk}_{h}_{k}",
                               tag="yo", bufs=4)
            nc.vector.tensor_add(yo[:, hsl], y_ps[:, hsl], xb_sb[k][:, isl])
            # ACT-ring triggers are only free once the exp stream is done
            eng = nc.scalar if (last and k == 1) else nc.sync
            eng.dma_start(out=yout[k * P:(k + 1) * P, isl], in_=yo[:, hsl])


def _emit_block_tail(nc, pools, blk, yout):
    ps_pool = pools["ps"]
    onesP_bf = pools["onesP_bf"]
    acc = pools["acc"][blk]
    den_ps = ps_pool.tile([P, 512], F32, name=f"den_ps{blk}", tag="pp",
                          bufs=2)
    nc.tensor.matmul(den_ps[:], onesP_bf[:], acc[:, 0:512],
                     start=True, stop=True)
    _emit_tail_finish(nc, pools, blk, yout, den_ps)


def _kernel_body(ctx, tc, ins, yout):
    nc = tc.nc
    xin, thw, phw, gw, wef, tb, bfin = (
        ins["xin"], ins["thw"], ins["phw"], ins["gw"], ins["wef"],
        ins["tb"], ins["bfin"])

    consts = ctx.enter_context(tc.tile_pool(name="consts", bufs=1))
    big = ctx.enter_context(tc.tile_pool(name="big", bufs=1))

    # ---- x load: fp16, 1024-col double-slices, both HWDGE rings ----
    x_sb = [big.tile([P, N], F16, name=f"x_sb{k}") for k in range(2)]

    def xdma(T, k):
        tsl = slice(T * 1024, (T + 1) * 1024)
        eng = nc.sync if k == 0 else nc.scalar
        eng.dma_start(out=x_sb[k][:, tsl], in_=xin[k * P:(k + 1) * P, tsl])

    # projection weights go first as small pieces on the ACT ring (they
    # gate the first matmul); x cols 0-511 go as 256-col pieces across
    # queues (per-queue BW ~22GB/s makes piece size the landing time)
    thw_sb = consts.tile([P, C], F16, name="thw_sb")
    phw_sb = consts.tile([P, C], F16, name="phw_sb")
    gw_sb = [consts.tile([P, IC], F16, name=f"gw_sb{k}") for k in range(2)]
    tb_sb = consts.tile([P, 1], F32, name="tb_sb")
    for k in range(2):
        nc.scalar.dma_start(out=thw_sb[:, k * P:(k + 1) * P],
                            in_=thw[k * P:(k + 1) * P, :])
    for q in range(2):
        for k in range(2):
            tsl = slice(q * 256, (q + 1) * 256)
            eng = nc.sync if k == 0 else nc.scalar
            eng.dma_start(out=x_sb[k][:, tsl],
                          in_=xin[k * P:(k + 1) * P, tsl])
    for k in range(2):
        nc.sync.dma_start(out=gw_sb[k][:], in_=gw[k * P:(k + 1) * P, :])
        nc.scalar.dma_start(out=phw_sb[:, k * P:(k + 1) * P],
                            in_=phw[k * P:(k + 1) * P, :])
    nc.sync.dma_start(out=tb_sb[:], in_=tb[:, None])
    for k in range(2):
        tsl = slice(512, 1024)
        eng = nc.sync if k == 0 else nc.scalar
        eng.dma_start(out=x_sb[k][:, tsl], in_=xin[k * P:(k + 1) * P, tsl])
    # preload the exp table set before the late x triggers queue up on
    # the ACT ring; tb_sb is already in flight on this ring
    exdum = consts.tile([P, 1], F32, name="exdum")
    nc.scalar.activation(exdum[:], tb_sb[:], AF.Exp)
    wef_hf = consts.tile([P, C], F16, name="wef_hf")
    nc.sync.dma_start(out=wef_hf[:], in_=wef[:, :])
    bfin_sb = consts.tile([P, 2], F32, name="bfin_sb")
    nc.sync.dma_start(out=bfin_sb[:], in_=bfin.rearrange("(k p) -> p k", p=P))
    for T in range(1, 4):
        for k in range(2):
            xdma(T, k)
    onesP_bf = consts.tile([P, P], BF16, name="onesP_bf")
    nc.vector.memset(onesP_bf[:], 1.0)

    theta_sb = big.tile([P, HALF], F16, name="theta_sb")
    phi_sb = big.tile([P, N], F16, name="phi_sb")
    gT_bf = big.tile([P, N], BF16, name="gT_bf")
    xb_sb = [big.tile([P, HALF], F16, name=f"xb_sb{k}") for k in range(2)]

    # ---- single PSUM pool, tagged slots (8 banks total):
    #   sc 2x[128,1024]=4, att 2x[128,512]=2, pp 2x[128,512]=2
    ps_pool = ctx.enter_context(tc.tile_pool(name="ps", bufs=1, space="PSUM"))
    pools = {
        "ps": ps_pool,
        "ex": ctx.enter_context(tc.tile_pool(name="ex", bufs=8)),
        "accp": ctx.enter_context(tc.tile_pool(name="accp", bufs=2)),
        "rec": ctx.enter_context(tc.tile_pool(name="rec", bufs=2)),
        "theta_sb": theta_sb, "phi_sb": phi_sb, "gT_bf": gT_bf,
        "onesP_bf": onesP_bf, "wef_hf": wef_hf, "xb_sb": xb_sb,
        "att_ps": {}, "acc": {}, "ex_sbs": {},
        "pend_tt": [], "pend_av": [], "tail_due": [],
    }

    def gt_chunks(t):
        # 4 chunks land side-by-side in one PSUM tile -> one cast per slice
        pst = ps_pool.tile([P, 512], F32, name=f"gt_ps{t}", tag="pp",
                           bufs=2)
        for i in range(4):
            jc = 4 * t + i
            jsl = slice(jc * P, (jc + 1) * P)
            for k in range(2):
                nc.tensor.matmul(pst[:, i * P:(i + 1) * P],
                                 x_sb[k][:, jsl], gw_sb[k][:],
                                 start=k == 0, stop=k == 1)
        nc.vector.tensor_copy(gT_bf[:, t * 512:(t + 1) * 512], pst[:])

    def proj(t):
        tsl = slice(t * 512, (t + 1) * 512)
        if t < NBLK:
            ps = ps_pool.tile([P, 512], F32, name=f"th_ps{t}", tag="pp",
                              bufs=2)
            for k in range(2):
                nc.tensor.matmul(ps[:], thw_sb[:, k * P:(k + 1) * P],
                                 x_sb[k][:, tsl],
                                 start=(k == 0), stop=(k == 1))
            nc.vector.tensor_scalar_add(theta_sb[:, tsl], ps[:], tb_sb[:])
        ps = ps_pool.tile([P, 512], F32, name=f"ph_ps{t}", tag="pp",
                          bufs=2)
        for k in range(2):
            nc.tensor.matmul(ps[:], phw_sb[:, k * P:(k + 1) * P],
                             x_sb[k][:, tsl],
                             start=(k == 0), stop=(k == 1))
        nc.vector.tensor_copy(phi_sb[:, tsl], ps[:])

    # ---- phase 1: slice-pipelined projections + gT (same-slice PSUM
    # rotation), interleaved with blocks 0 AND 1 of the attention (4
    # groups per x slice) so the PE is fed as soon as each slice lands.
    proj(0)
    gt_chunks(0)
    for t in range(1, 8):
        proj(t)
        gt_chunks(t)
        _flush_tt(nc, pools)
        for blkg in (0, 1):
            for gg in (2 * (t - 1), 2 * (t - 1) + 1):
                _emit_group(nc, pools, blkg, gg, yout, defer_tt=True)
    _flush_tt(nc, pools)
    for grp in range(14, NGRP):
        _emit_group(nc, pools, 0, grp, yout)
    for grp in range(14, NGRP):
        _emit_group(nc, pools, 1, grp, yout)
    for k in range(2):
        nc.vector.tensor_scalar_add(xb_sb[k][:], x_sb[k][:, 0:HALF],
                                    bfin_sb[:, k:k + 1])

    # ---- remaining i-blocks; AV consumption and tails flow through the
    # global deferred queue so block boundaries stay busy on both engines.
    for blk in range(2, NBLK):
        for grp in range(NGRP):
            _emit_group(nc, pools, blk, grp, yout)

    # drain: the last block's pairs 6/7 feed the denominator ones-matmuls
    # directly, interleaved with the leftover AVs in data-arrival order so
    # the reciprocal can start right after the final exp
    last = NBLK - 1
    ones = pools["onesP_bf"]
    ex6 = pools["ex_sbs"][(last, NPAIR - 2)]
    ex7 = pools["ex_sbs"][(last, NPAIR - 1)]
    den_ps = pools["ps"].tile([P, 512], F32, name=f"den_ps{last}",
                              tag="pp", bufs=2)
    _pop_av(nc, pools, yout)                       # AV(last, 12)
    nc.tensor.matmul(den_ps[:], ones[:], pools["acc"][last][:, 0:512],
                     start=True, stop=False)
    for c in range(4):                             # chunks of groups 12,13
        nc.tensor.matmul(den_ps[:], ones[:], ex6[:, c * 512:(c + 1) * 512],
                         start=False, stop=False)
    _pop_av(nc, pools, yout)                       # AV(last, 13)
    _pop_av(nc, pools, yout)                       # AV(last, 14)
    for c in range(2):                             # chunks of group 14
        nc.tensor.matmul(den_ps[:], ones[:], ex7[:, c * 512:(c + 1) * 512],
                         start=False, stop=False)
    _pop_av(nc, pools, yout)                       # AV(last, 15)
    for c in range(2, 4):                          # chunks of group 15
        nc.tensor.matmul(den_ps[:], ones[:], ex7[:, c * 512:(c + 1) * 512],
                         start=False, stop=c == 3)
    pools["tail_due"] = []
    _emit_tail_finish(nc, pools, last, yout, den_ps)


_CACHE = {}


def _build():
    if "nc" in _CACHE:
        return _CACHE["nc"]
    nc = bacc.Bacc("TRN2", target_bir_lowering=False, debug=False,
                   enable_asserts=False, num_devices=1)
    ins = {
        "xin": nc.dram_tensor("xin", [C, N], F16, kind="ExternalInput").ap(),
        "thw": nc.dram_tensor("thw", [C, IC], F16, kind="ExternalInput").ap(),
        "phw": nc.dram_tensor("phw", [C, IC], F16, kind="ExternalInput").ap(),
        "gw": nc.dram_tensor("gw", [C, IC], F16, kind="ExternalInput").ap(),
        "wef": nc.dram_tensor("wef", [IC, C], F16, kind="ExternalInput").ap(),
        "tb": nc.dram_tensor("tb", [IC], F32, kind="ExternalInput").ap(),
        "bfin": nc.dram_tensor("bfin", [C], F32, kind="ExternalInput").ap(),
    }
    yout = nc.dram_tensor("yout", [C, HALF], F32, kind="ExternalOutput").ap()
    with tile.TileContext(nc) as tc:
        with ExitStack() as ctx:
            _kernel_body(ctx, tc, ins, yout)
    nc.compile()
    _CACHE["nc"] = nc
    return nc


def _host_prepare(inputs):
    """Host-side folds + per-core input maps."""
    ii = {k: np.ascontiguousarray(np.asarray(v, dtype=np.float32))
          for k, v in inputs.items()}
    inv = ii["bn_gamma"] / np.sqrt(ii["bn_var"] + BN_EPS)
    w_eff = ii["w_w"] * inv[:, None]                       # [C, IC]
    b_final = (w_eff @ ii["g_b"] + ii["w_b"] * inv
               + ii["bn_beta"] - ii["bn_mean"] * inv)      # [C]
    f16 = np.float16
    shared = {
        "thw": np.ascontiguousarray(ii["theta_w"].T.astype(f16)),
        "phw": np.ascontiguousarray(ii["phi_w"].T.astype(f16)),
        "gw": np.ascontiguousarray(ii["g_w"].T.astype(f16)),
        "wef": np.ascontiguousarray(w_eff.T.astype(f16)),  # [IC, C]
        "tb": ii["theta_b"],
        "bfin": np.ascontiguousarray(b_final),
    }
    x = ii["x"].reshape(B, C, N)
    in_maps = []
    for core in range(NCORES):
        b, h = divmod(core, 2)
        own = x[b][:, h * HALF:(h + 1) * HALF]
        oth = x[b][:, (1 - h) * HALF:(2 - h) * HALF]
        xin = np.ascontiguousarray(
            np.concatenate([own, oth], axis=1).astype(f16))
        in_maps.append({"xin": xin, **shared})
    return in_maps


def _gather(results, x_dtype):
    out = np.empty((B, C, N), dtype=np.float32)
    for core in range(NCORES):
        b, h = divmod(core, 2)
        out[b][:, h * HALF:(h + 1) * HALF] = results[core]["yout"]
    return out.reshape(B, C, H, W).astype(x_dtype, copy=False)


def kernel(**inputs):
    nc = _build()
    in_maps = _host_prepare(inputs)
    res = run_bass_kernel_spmd(nc, in_maps, core_ids=list(range(NCORES)))
    return _gather(res.results, np.asarray(inputs["x"]).dtype)


# revision 33
# speedup vs baseline: 1.1962x; 1.0053x over previous
# NonLocalBlock Trainium2 Bass kernel (v6).
#
# Reference computation (per batch b):
#   theta = theta_w @ X + theta_b          [IC, N]   (X = x[b] as [C, N])
#   phi   = phi_w   @ X + phi_b            [IC, N]
#   g     = g_w     @ X + g_b              [IC, N]
#   attn  = softmax_j(theta^T phi)         [N, N]
#   att   = g @ attn^T                     [IC, N]
#   y     = BN(w_w @ att + w_b) + x
#
# Math folds used on device (validated vs reference):
#   - phi bias drops out of softmax entirely (adds an i-only constant).
#   - g bias folds into the final bias because attn rows sum to 1.
#   - BN is affine: fold into w_eff = inv*w_w and b_final.
#   - scores bounded (|s| < 50) so exp() needs no max-subtraction.
#
# Sharding: 8 cores = 4 batches x 2 row-halves (pure SPMD).
#
# Pipeline structure (v6):
#   - x and all projection weights ship as fp16: half the HBM traffic
#     (x = 2MB in 8 big DMAs), FWL on every weight load, and |x| < 6 so
#     fp16 rounding (~5e-4) is negligible against the 2e-2 gate.
#   - scores are computed transposed ([j,i]) in fp16 (phi stationary,
#     theta moving) so exp feeds att = g @ attn^T directly.
#   - exp is written as bf16; the softmax denominator is accumulated by
#     the DVE (one [128,2048] bf16 add per exp tile, 2 elem/cyc/lane),
#     folded to one chunk-slot, then a single ones-matmul per block does
#     the cross-partition reduction. The last block instead feeds its
#     final exp tile straight into ones-matmuls and runs a half-width
#     normalize/W/store pipeline to shorten the drain.
#   - AV consumption runs through a global 4-group-deferred queue that
#     crosses block boundaries, so block-end leftovers interleave with
#     the next block's scores instead of starving the ACT engine.
#   - gT is produced directly as x_chunk^T @ g_w^T, emitted in the same
#     slice as its projection (same-slice PSUM rotation, no convoy).
#   - The exp-table preload is emitted before the late x DMA triggers on
#     the ACT ring so the table is resident before the first real exp.
#   - PSUM: sc 2x[128,1024]=4 banks, att 2x[128,512], pp 2x[128,512].

from contextlib import ExitStack

import numpy as np

import concourse.bass as bass
import concourse.tile as tile
from concourse import bacc, mybir
from concourse.bass_utils import run_bass_kernel_spmd

F32 = mybir.dt.float32
F32R = mybir.dt.float32r
BF16 = mybir.dt.bfloat16
F16 = mybir.dt.float16
AF = mybir.ActivationFunctionType

B, C, IC = 4, 256, 128
H = W = 64
N = H * W            # 4096
HALF = N // 2        # 2048 rows of attention per core
P = 128
NCORES = 8
NBLK = HALF // 512   # 4 i-blocks of 512
NCH = N // P         # 32 j-chunks of 128
NGRP = NCH // 2      # 16 groups of 2 chunks per i-block
NPAIR = NGRP // 2    # 8 exp-tile pairs per i-block
DEFER = 4            # consume exp output this many groups late (global)
BN_EPS = 1e-5


def _emit_consume(nc, pools, blk, grp):
    """AV matmuls for group `grp` of block `blk` (bf16)."""
    att_ps = pools["att_ps"][blk]
    gT_bf = pools["gT_bf"]
    ex2 = pools["ex_sbs"][(blk, grp // 2)]
    off = (grp % 2) * 1024
    for c in range(2):
        jc = grp * 2 + c
        nc.tensor.matmul(
            att_ps[:], gT_bf[:, jc * P:(jc + 1) * P],
            ex2[:, off + c * 512:off + (c + 1) * 512],
            start=jc == 0, stop=jc == NCH - 1)


def _emit_den_fold(nc, pools, blk):
    """Fold acc's 4 chunk-slots down to one (slot 0) on the DVE."""
    acc = pools["acc"][blk]
    nc.vector.tensor_add(acc[:, 0:1024], acc[:, 0:1024], acc[:, 1024:2048])
    nc.vector.tensor_add(acc[:, 0:512], acc[:, 0:512], acc[:, 512:1024])


def _emit_den_tt(nc, pools, blk, pair):
    """Accumulate one exp tile (2 groups) into the block's bf16 acc."""
    acc = pools["acc"][blk]
    ex2 = pools["ex_sbs"][(blk, pair)]
    if pair == 0:
        nc.vector.tensor_copy(acc[:], ex2[:])
    else:
        nc.vector.tensor_add(acc[:], acc[:], ex2[:])
    last = NBLK - 1
    if (blk < last and pair == NPAIR - 1) or (blk == last and
                                              pair == NPAIR - 3):
        _emit_den_fold(nc, pools, blk)


def _flush_tt(nc, pools):
    for blk, pair in pools["pend_tt"]:
        _emit_den_tt(nc, pools, blk, pair)
    pools["pend_tt"] = []


def _pop_av(nc, pools, yout):
    blk, grp = pools["pend_av"].pop(0)
    _emit_consume(nc, pools, blk, grp)
    if grp == NGRP - 1:
        pools["tail_due"].append(blk)


def _emit_due_tails(nc, pools, yout):
    for blk in pools["tail_due"]:
        _emit_block_tail(nc, pools, blk, yout)
    pools["tail_due"] = []


def _emit_group(nc, pools, blk, grp, yout, defer_tt=False):
    """AV (deferred), then scores + exp for one [128,1024] group."""
    ps_pool, ex_pool, acc_pool = pools["ps"], pools["ex"], pools["accp"]
    theta_sb, phi_sb = pools["theta_sb"], pools["phi_sb"]
    isl = slice(blk * 512, (blk + 1) * 512)
    if grp == 0:
        pools["att_ps"][blk] = ps_pool.tile(
            [P, 512], F32, name=f"att_ps{blk}", tag="att", bufs=2)
        pools["acc"][blk] = acc_pool.tile(
            [P, 2048], BF16, name=f"acc{blk}", tag="acc", bufs=2)
    if grp % 2 == 0:
        pools["ex_sbs"][(blk, grp // 2)] = ex_pool.tile(
            [P, 2048], BF16, name=f"ex{blk}_{grp // 2}", tag="ex", bufs=8)
    if len(pools["pend_av"]) >= DEFER:
        _pop_av(nc, pools, yout)
    _emit_due_tails(nc, pools, yout)
    sc_ps = ps_pool.tile([P, 1024], F32, name=f"sc{blk}_{grp}", tag="sc",
                         bufs=2)
    for c in range(2):
        jc = grp * 2 + c
        nc.tensor.matmul(
            sc_ps[:, c * 512:(c + 1) * 512],
            phi_sb[:, jc * P:(jc + 1) * P],
            theta_sb[:, isl],
            start=True, stop=True)
    ex2 = pools["ex_sbs"][(blk, grp // 2)]
    off = (grp % 2) * 1024
    nc.scalar.activation(ex2[:, off:off + 1024], sc_ps[:], AF.Exp)
    pools["pend_av"].append((blk, grp))
    # last block: pairs 6 and 7 skip the DVE accumulate; the drain feeds
    # them straight into the denominator ones-matmuls
    if grp % 2 == 1 and not (blk == NBLK - 1 and grp >= NGRP - 4):
        if defer_tt:
            pools["pend_tt"].append((blk, grp // 2))
        else:
            _emit_den_tt(nc, pools, blk, grp // 2)


def _emit_tail_finish(nc, pools, blk, yout, den_ps):
    """Reciprocal, normalize, W projection, bias+residual, store."""
    ps_pool, rec_pool = pools["ps"], pools["rec"]
    wef_hf, xb_sb = pools["wef_hf"], pools["xb_sb"]
    att_ps = pools["att_ps"][blk]
    last = blk == NBLK - 1

    recb = rec_pool.tile([P, 512], F32, name=f"recb{blk}", tag="recb")
    nc.vector.reciprocal_approx_fast(out=recb[:], in_=den_ps[:])
    halves = (slice(0, 256), slice(256, 512)) if last else (slice(0, 512),)
    attn_sb = rec_pool.tile([P, 512], F16, name=f"attn{blk}", tag="attn")
    for h, hsl in enumerate(halves):
        isl = slice(blk * 512 + hsl.start, blk * 512 + hsl.stop)
        nc.vector.tensor_mul(attn_sb[:, hsl], att_ps[:, hsl], recb[:, hsl])
        for k in range(2):
            y_ps = ps_pool.tile([P, 512], F32, name=f"y{blk}_{h}_{k}",
                                tag="pp", bufs=2)
            nc.tensor.matmul(
                y_ps[:, hsl], wef_hf[:, k * P:(k + 1) * P], attn_sb[:, hsl],
                start=True, stop=True)
            yo = rec_pool.tile([P, 512], F32, name=f"yo{blk}_{h}_{k}",
                               tag="yo", bufs=4)
            nc.vector.tensor_add(yo[:, hsl], y_ps[:, hsl], xb_sb[k][:, isl])
            # ACT-ring triggers are only free once the exp stream is done
            eng = nc.scalar if (last and k == 1) else nc.sync
            eng.dma_start(out=yout[k * P:(k + 1) * P, isl], in_=yo[:, hsl])


def _emit_block_tail(nc, pools, blk, yout):
    ps_pool = pools["ps"]
    onesP_bf = pools["onesP_bf"]
    acc = pools["acc"][blk]
    den_ps = ps_pool.tile([P, 512], F32, name=f"den_ps{blk}", tag="pp",
                          bufs=2)
    nc.tensor.matmul(den_ps[:], onesP_bf[:], acc[:, 0:512],
                     start=True, stop=True)
    _emit_tail_finish(nc, pools, blk, yout, den_ps)


def _kernel_body(ctx, tc, ins, yout):
    nc = tc.nc
    xin, thw, phw, gw, wef, tb, bfin = (
        ins["xin"], ins["thw"], ins["phw"], ins["gw"], ins["wef"],
        ins["tb"], ins["bfin"])

    consts = ctx.enter_context(tc.tile_pool(name="consts", bufs=1))
    big = ctx.enter_context(tc.tile_pool(name="big", bufs=1))

    # ---- x load: fp16, 1024-col double-slices, both HWDGE rings ----
    x_sb = [big.tile([P, N], F16, name=f"x_sb{k}") for k in range(2)]

    def xdma(T, k):
        tsl = slice(T * 1024, (T + 1) * 1024)
        eng = nc.sync if k == 0 else nc.scalar
        eng.dma_start(out=x_sb[k][:, tsl], in_=xin[k * P:(k + 1) * P, tsl])

    # projection weights go first as small pieces on the ACT ring (they
    # gate the first matmul); x cols 0-511 go as 256-col pieces across
    # queues (per-queue BW ~22GB/s makes piece size the landing time)
    thw_sb = consts.tile([P, C], F16, name="thw_sb")
    phw_sb = consts.tile([P, C], F16, name="phw_sb")
    gw_sb = [consts.tile([P, IC], F16, name=f"gw_sb{k}") for k in range(2)]
    tb_sb = consts.tile([P, 1], F32, name="tb_sb")
    for k in range(2):
        nc.scalar.dma_start(out=thw_sb[:, k * P:(k + 1) * P],
                            in_=thw[k * P:(k + 1) * P, :])
    for q in range(2):
        for k in range(2):
            tsl = slice(q * 256, (q + 1) * 256)
            eng = nc.sync if k == 0 else nc.scalar
            eng.dma_start(out=x_sb[k][:, tsl],
                          in_=xin[k * P:(k + 1) * P, tsl])
    for k in range(2):
        nc.sync.dma_start(out=gw_sb[k][:], in_=gw[k * P:(k + 1) * P, :])
        nc.scalar.dma_start(out=phw_sb[:, k * P:(k + 1) * P],
                            in_=phw[k * P:(k + 1) * P, :])
    nc.sync.dma_start(out=tb_sb[:], in_=tb[:, None])
    for k in range(2):
        tsl = slice(512, 1024)
        eng = nc.sync if k == 0 else nc.scalar
        eng.dma_start(out=x_sb[k][:, tsl], in_=xin[k * P:(k + 1) * P, tsl])
    # preload the exp table set before the late x triggers queue up on
    # the ACT ring; tb_sb is already in flight on this ring
    exdum = consts.tile([P, 1], F32, name="exdum")
    nc.scalar.activation(exdum[:], tb_sb[:], AF.Exp)
    wef_hf = consts.tile([P, C], F16, name="wef_hf")
    nc.sync.dma_start(out=wef_hf[:], in_=wef[:, :])
    bfin_sb = consts.tile([P, 2], F32, name="bfin_sb")
    nc.sync.dma_start(out=bfin_sb[:], in_=bfin.rearrange("(k p) -> p k", p=P))
    for T in range(1, 4):
        for k in range(2):
            xdma(T, k)
    onesP_bf = consts.tile([P, P], BF16, name="onesP_bf")
    nc.vector.memset(onesP_bf[:], 1.0)

    theta_sb = big.tile([P, HALF], F16, name="theta_sb")
    phi_sb = big.tile([P, N], F16, name="phi_sb")
    gT_bf = big.tile([P, N], BF16, name="gT_bf")
    xb_sb = [big.tile([P, HALF], F16, name=f"xb_sb{k}") for k in range(2)]

    # ---- single PSUM pool, tagged slots (8 banks total):
    #   sc 2x[128,1024]=4, att 2x[128,512]=2, pp 2x[128,512]=2
    ps_pool = ctx.enter_context(tc.tile_pool(name="ps", bufs=1, space="PSUM"))
    pools = {
        "ps": ps_pool,
        "ex": ctx.enter_context(tc.tile_pool(name="ex", bufs=8)),
        "accp": ctx.enter_context(tc.tile_pool(name="accp", bufs=2)),
        "rec": ctx.enter_context(tc.tile_pool(name="rec", bufs=2)),
        "theta_sb": theta_sb, "phi_sb": phi_sb, "gT_bf": gT_bf,
        "onesP_bf": onesP_bf, "wef_hf": wef_hf, "xb_sb": xb_sb,
        "att_ps": {}, "acc": {}, "ex_sbs": {},
        "pend_tt": [], "pend_av": [], "tail_due": [],
    }

    def gt_chunks(t):
        # 4 chunks land side-by-side in one PSUM tile -> one cast per slice
        pst = ps_pool.tile([P, 512], F32, name=f"gt_ps{t}", tag="pp",
                           bufs=2)
        for i in range(4):
            jc = 4 * t + i
            jsl = slice(jc * P, (jc + 1) * P)
            for k in range(2):
                nc.tensor.matmul(pst[:, i * P:(i + 1) * P],
                                 x_sb[k][:, jsl], gw_sb[k][:],
                                 start=k == 0, stop=k == 1)
        nc.vector.tensor_copy(gT_bf[:, t * 512:(t + 1) * 512], pst[:])

    def proj(t):
        tsl = slice(t * 512, (t + 1) * 512)
        if t < NBLK:
            ps = ps_pool.tile([P, 512], F32, name=f"th_ps{t}", tag="pp",
                              bufs=2)
            for k in range(2):
                nc.tensor.matmul(ps[:], thw_sb[:, k * P:(k + 1) * P],
                                 x_sb[k][:, tsl],
                                 start=(k == 0), stop=(k == 1))
            nc.vector.tensor_scalar_add(theta_sb[:, tsl], ps[:], tb_sb[:])
        ps = ps_pool.tile([P, 512], F32, name=f"ph_ps{t}", tag="pp",
                          bufs=2)
        for k in range(2):
            nc.tensor.matmul(ps[:], phw_sb[:, k * P:(k + 1) * P],
                             x_sb[k][:, tsl],
                             start=(k == 0), stop=(k == 1))
        nc.vector.tensor_copy(phi_sb[:, tsl], ps[:])

    # ---- phase 1: slice-pipelined projections + gT (same-slice PSUM
    # rotation), interleaved with blocks 0 AND 1 of the attention (4
    # groups per x slice) so the PE is fed as soon as each slice lands.
    proj(0)
    gt_chunks(0)
    for t in range(1, 8):
        proj(t)
        gt_chunks(t)
        _flush_tt(nc, pools)
        for blkg in (0, 1):
            for gg in (2 * (t - 1), 2 * (t - 1) + 1):
                _emit_group(nc, pools, blkg, gg, yout, defer_tt=True)
    _flush_tt(nc, pools)
    for grp in range(14, NGRP):
        _emit_group(nc, pools, 0, grp, yout)
    for grp in range(14, NGRP):
        _emit_group(nc, pools, 1, grp, yout)
    for k in range(2):
        nc.vector.tensor_scalar_add(xb_sb[k][:], x_sb[k][:, 0:HALF],
                                    bfin_sb[:, k:k + 1])

    # ---- remaining i-blocks; AV consumption and tails flow through the
    # global deferred queue so block boundaries stay busy on both engines.
    for blk in range(2, NBLK):
        for grp in range(NGRP):
            _emit_group(nc, pools, blk, grp, yout)

    # drain: the last block's pairs 6/7 feed the denominator ones-matmuls
    # directly, interleaved with the leftover AVs in data-arrival order so
    # the reciprocal can start right after the final exp
    last = NBLK - 1
    ones = pools["onesP_bf"]
    ex6 = pools["ex_sbs"][(last, NPAIR - 2)]
    ex7 = pools["ex_sbs"][(last, NPAIR - 1)]
    den_ps = pools["ps"].tile([P, 512], F32, name=f"den_ps{last}",
                              tag="pp", bufs=2)
    nc.tensor.matmul(den_ps[:], ones[:], pools["acc"][last][:, 0:512],
                     start=True, stop=False)
    for c in range(4):                             # chunks of groups 12,13
        nc.tensor.matmul(den_ps[:], ones[:], ex6[:, c * 512:(c + 1) * 512],
                         start=False, stop=False)
    _pop_av(nc, pools, yout)                       # AV(last, 12)
    _pop_av(nc, pools, yout)                       # AV(last, 13)
    for c in range(2):                             # chunks of group 14
        nc.tensor.matmul(den_ps[:], ones[:], ex7[:, c * 512:(c + 1) * 512],
                         start=False, stop=False)
    _pop_av(nc, pools, yout)                       # AV(last, 14)
    for c in range(2, 4):                          # chunks of group 15
        nc.tensor.matmul(den_ps[:], ones[:], ex7[:, c * 512:(c + 1) * 512],
                         start=False, stop=c == 3)
    _pop_av(nc, pools, yout)                       # AV(last, 15)
    pools["tail_due"] = []
    _emit_tail_finish(nc, pools, last, yout, den_ps)


_CACHE = {}


def _build():
    if "nc" in _CACHE:
        return _CACHE["nc"]
    nc = bacc.Bacc("TRN2", target_bir_lowering=False, debug=False,
                   enable_asserts=False, num_devices=1)
    ins = {
        "xin": nc.dram_tensor("xin", [C, N], F16, kind="ExternalInput").ap(),
        "thw": nc.dram_tensor("thw", [C, IC], F16, kind="ExternalInput").ap(),
        "phw": nc.dram_tensor("phw", [C, IC], F16, kind="ExternalInput").ap(),
        "gw": nc.dram_tensor("gw", [C, IC], F16, kind="ExternalInput").ap(),
        "wef": nc.dram_tensor("wef", [IC, C], F16, kind="ExternalInput").ap(),
        "tb": nc.dram_tensor("tb", [IC], F32, kind="ExternalInput").ap(),
        "bfin": nc.dram_tensor("bfin", [C], F32, kind="ExternalInput").ap(),
    }
    yout = nc.dram_tensor("yout", [C, HALF], F32, kind="ExternalOutput").ap()
    with tile.TileContext(nc) as tc:
        with ExitStack() as ctx:
            _kernel_body(ctx, tc, ins, yout)
    nc.compile()
    _CACHE["nc"] = nc
    return nc


def _host_prepare(inputs):
    """Host-side folds + per-core input maps."""
    ii = {k: np.ascontiguousarray(np.asarray(v, dtype=np.float32))
          for k, v in inputs.items()}
    inv = ii["bn_gamma"] / np.sqrt(ii["bn_var"] + BN_EPS)
    w_eff = ii["w_w"] * inv[:, None]                       # [C, IC]
    b_final = (w_eff @ ii["g_b"] + ii["w_b"] * inv
               + ii["bn_beta"] - ii["bn_mean"] * inv)      # [C]
    f16 = np.float16
    shared = {
        "thw": np.ascontiguousarray(ii["theta_w"].T.astype(f16)),
        "phw": np.ascontiguousarray(ii["phi_w"].T.astype(f16)),
        "gw": np.ascontiguousarray(ii["g_w"].T.astype(f16)),
        "wef": np.ascontiguousarray(w_eff.T.astype(f16)),  # [IC, C]
        "tb": ii["theta_b"],
        "bfin": np.ascontiguousarray(b_final),
    }
    x = ii["x"].reshape(B, C, N)
    in_maps = []
    for core in range(NCORES):
        b, h = divmod(core, 2)
        own = x[b][:, h * HALF:(h + 1) * HALF]
        oth = x[b][:, (1 - h) * HALF:(2 - h) * HALF]
        xin = np.ascontiguousarray(
            np.concatenate([own, oth], axis=1).astype(f16))
        in_maps.append({"xin": xin, **shared})
    return in_maps


def _gather(results, x_dtype):
    out = np.empty((B, C, N), dtype=np.float32)
    for core in range(NCORES):
        b, h = divmod(core, 2)
        out[b][:, h * HALF:(h + 1) * HALF] = results[core]["yout"]
    return out.reshape(B, C, H, W).astype(x_dtype, copy=False)


def kernel(**inputs):
    nc = _build()
    in_maps = _host_prepare(inputs)
    res = run_bass_kernel_spmd(nc, in_maps, core_ids=list(range(NCORES)))
    return _gather(res.results, np.asarray(inputs["x"]).dtype)
